# revision 19
# baseline (speedup 1.0000x reference)
"""Causal attention kernel for Trainium2, 8 NeuronCores, sequence-parallel.

Reference computation (T=4096, D=1024, fp32):
    q = x @ Wqk; logits = q @ x.T (causal masked); attn = softmax(logits)
    out = (attn @ x) @ Wov

Causal load balancing under one SPMD program: the 32 query row-tiles of 128
are assigned to cores as {c, 15-c, 16+c, 31-c} and host-permuted into 4
local "slots" ordered by visibility class. Slot m processes a fixed key
budget of 8*(m+1) key-tiles (keys in natural order, prefix [0, 1024*(m+1))),
which covers every core's visible range in that class. Causality inside the
budget is enforced by a host-provided additive mask (0 / -60000) that also
carries the diagonal triangle, so the program is core-independent while
skipping 37.5% of the score/AV matmul work.

Matmul precision: fp16 inputs (x, Wqk, Wov, attn) with fp32 PSUM
accumulation; q and o1 kept in fp16 on-chip. Softmax row max subtracted in
fp32; attn stored fp16 for the DMA-xbar transposes and AV.

Scheduling notes: input DMAs are issued in consumption order (xqt/wqk for
phase A first, then keys/masks); phase B runs slots largest-first so the
exp/transpose pipeline drains during B and phase E can start right after;
tiles are split per dependency unit (per-kg keys, per-chunk attn-transpose,
per-d o1) to keep cross-engine waits granular.
"""

import sys

sys.path.insert(0, "/opt/trn_rl_repo")

import numpy as np

import concourse.tile as tile
from concourse import bacc, mybir
from concourse.bass_utils import run_bass_kernel_spmd

T = 4096
D = 1024
NCORES = 8
RQ = T // NCORES  # 512 query rows per core
KC = D // 128  # 8 contraction chunks
NEG16 = -57344.0  # exactly representable in fp8e5m2

BKT = [8, 16, 24, 32]  # key tiles (128) processed per slot
BG = [b // 4 for b in BKT]  # 512-wide key groups per slot
OFFK = [0, 1024, 3072, 6144]  # slot column offsets in ragged score layout
STOT = 10240  # total score/mask columns
MPOFF = [0, 2, 6, 12]  # mpart offsets (prefix of BG)
NCH = [b // 8 for b in BKT]  # 1024-wide exp chunks per slot: 1,2,3,4
LQOFF = [0, 1, 3, 6]  # lq offsets (prefix of NCH)

f32 = mybir.dt.float32
f16 = mybir.dt.float16
f8 = mybir.dt.float8e5
f8e4 = mybir.dt.float8e4
DR = mybir.MatmulPerfMode.DoubleRow


def _build_nc():
    nc = bacc.Bacc(
        "TRN2", target_bir_lowering=False, debug=False, num_devices=NCORES
    )

    xqt_d = nc.dram_tensor("xqt", [D, RQ], f16, kind="ExternalInput").ap()
    xtp_d = nc.dram_tensor("xtp", [D, T], f16, kind="ExternalInput").ap()
    # x rows in fp8 hi/lo, host-permuted so partition p of a 256-row
    # pairgroup holds rows 2p and 2p+1 (DoubleRow contraction pairs)
    xph_d = nc.dram_tensor("xph", [T, D], f8e4, kind="ExternalInput").ap()
    xpl_d = nc.dram_tensor("xpl", [T, D], f8e4, kind="ExternalInput").ap()
    wqk_d = nc.dram_tensor("wqk", [D, D], f16, kind="ExternalInput").ap()
    wov_d = nc.dram_tensor("wov", [D, D], f16, kind="ExternalInput").ap()
    # causal mask, only the last 1024 keys of each slot's budget: for class
    # m (tiles 8m..8m+7) keys below 1024m are visible to every member core,
    # so only the final window carries the triangle / -inf region
    mask_d = nc.dram_tensor("mask", [128, 4096], f8, kind="ExternalInput").ap()
    out_d = nc.dram_tensor("out", [RQ, D], f16, kind="ExternalOutput").ap()

    with tile.TileContext(nc) as tc:
        # stack allocator: long-lived pools first
        consts = tc.alloc_tile_pool(name="consts", bufs=1)
        pt_pool = tc.alloc_tile_pool(name="ptpool", bufs=1)
        o1_pool = tc.alloc_tile_pool(name="o1pool", bufs=1)
        xpstream = tc.alloc_tile_pool(name="xpstream", bufs=3)
        p_pool = tc.alloc_tile_pool(name="ppool", bufs=2)
        s_pool = tc.alloc_tile_pool(name="spool", bufs=2)
        qt_pool = tc.alloc_tile_pool(name="qt", bufs=1)
        xtp_pool = tc.alloc_tile_pool(name="xtpp", bufs=1)
        mask_pool = tc.alloc_tile_pool(name="maskp", bufs=1)
        wqk_pool = tc.alloc_tile_pool(name="wqkp", bufs=1)
        xqt_pool = tc.alloc_tile_pool(name="xqtp", bufs=1)

        # stats scratch: negmax 0:4, lsum 4:8, recip 8:12, mpart 12:32, lq 32:42
        smalls = consts.tile([128, 48], f32, name="smalls")
        dum = consts.tile([128, 256], f16, name="dum")
        negmax = smalls[:, 0:4]
        lsum = smalls[:, 4:8]
        recip = smalls[:, 8:12]
        mpart = smalls[:, 12:32]
        lq = smalls[:, 32:42]

        # transposed attn, fp8, packed in key-PAIRS: the xbar moves 2-byte
        # granules, so attn fp8 is transposed as an fp16 view — partition p
        # of a 256-key pairgroup then holds keys (2p, 2p+1) as adjacent
        # bytes, exactly the [p, 2, q] moving layout DoubleRow contracts.
        # Ragged blocks: block c (pairgroups 4c..4c+3) has q-width
        # (4-c)*128 covering slots m >= c; byte offsets PTOFF.
        PTOFF = [0, 4096, 7168, 9216]  # byte prefix sums of 4*2*width(c)
        WC = [512, 384, 256, 128]  # q-cols per block
        ptall = pt_pool.tile([128, STOT], f8e4, name="ptall")

        def pt_view16(m, c):
            # [128, 4 pairgroups, 128] fp16 xbar-dst view of slot m, block c
            w = WC[c]
            span = ptall[:].bitcast(f16)[
                :, PTOFF[c] // 2 : PTOFF[c] // 2 + 4 * w
            ].rearrange("p (pg w) -> p pg w", pg=4)
            return span[:, :, (m - c) * 128 : (m - c) * 128 + 128]
        o1t = [o1_pool.tile([128, RQ], f16, name=f"o1t{d}") for d in range(KC)]
        qt_sb = qt_pool.tile([128, KC * RQ], f16, name="qt_sb")
        xtp_t = [
            xtp_pool.tile([128, KC * 512], f16, name=f"xtp{kg}")
            for kg in range(T // 512)
        ]
        mask_t = [
            mask_pool.tile([128, 1024], f8, name=f"mask{m}") for m in range(4)
        ]
        wqk_t = [
            wqk_pool.tile([128, KC * 256], f16, name=f"wqk{md2}")
            for md2 in range(KC // 2)
        ]
        xqt_sb = xqt_pool.tile([128, KC * RQ], f16, name="xqt_sb")

        # ---- input DMAs, issued in consumption order ---------------------
        def load_wqk(md2):
            nc.sync.dma_start(
                wqk_t[md2].rearrange("p (kc n) -> p kc n", kc=KC),
                wqk_d[:, md2 * 256 : (md2 + 1) * 256].rearrange(
                    "(kc p) n -> p kc n", p=128
                ),
            )

        def load_xtp(kg):
            nc.sync.dma_start(
                xtp_t[kg].rearrange("p (kc n) -> p kc n", kc=KC),
                xtp_d[:, kg * 512 : (kg + 1) * 512].rearrange(
                    "(kc p) n -> p kc n", p=128
                ),
            )

        def load_mask(m):
            nc.sync.dma_start(
                mask_t[m], mask_d[:, m * 1024 : (m + 1) * 1024]
            )

        nc.sync.dma_start(
            xqt_sb.rearrange("p (kc n) -> p kc n", kc=KC),
            xqt_d.rearrange("(kc p) n -> p kc n", p=128),
        )
        load_wqk(0)
        load_wqk(1)
        load_wqk(2)
        load_xtp(0)
        load_wqk(3)
        load_xtp(1)
        load_xtp(2)
        load_xtp(3)
        load_mask(3)
        for kg in range(4, 8):
            load_xtp(kg)
        load_mask(2)
        load_mask(1)
        load_mask(0)

        # PE p-state warmup: the tensor engine downclocks when idle and
        # takes ~3us to re-ramp. Keep it hot with throwaway matmuls into a
        # dedicated PSUM bank while input DMAs land / cross-engine deps
        # resolve. psW is allocated first so its WAR chains stay PE-internal.
        psW = tc.alloc_tile_pool(name="psW", bufs=1, space="PSUM")
        wps = psW.tile([128, 512], f32, name="wps")
        nc.gpsimd.memset(dum[:], 0.0)

        def warm(n):
            for _ in range(n):
                nc.tensor.matmul(
                    wps[:, 0:256], dum[:, 0:128], dum[:], start=True, stop=True
                )

        warm(18)

        # ---- Phase A: qT = (xq @ Wqk)^T  -> [D, RQ] fp16 -----------------
        with tc.tile_pool(name="psA", bufs=2, space="PSUM") as psA:
            for md2 in range(KC // 2):
                for h in range(2):
                    mtd = md2 * 2 + h
                    ps = psA.tile([128, RQ], f32, name="ps_qt")
                    for kc in range(KC):
                        nc.tensor.matmul(
                            ps[:],
                            wqk_t[md2][
                                :, kc * 256 + h * 128 : kc * 256 + h * 128 + 128
                            ],
                            xqt_sb[:, kc * RQ : (kc + 1) * RQ],
                            start=(kc == 0),
                            stop=(kc == KC - 1),
                        )
                    nc.vector.tensor_copy(
                        qt_sb[:, mtd * RQ : (mtd + 1) * RQ], ps[:]
                    )
        xqt_pool.release()
        wqk_pool.release()

        # ---- Phase B: per-slot scores + mask + softmax stats + exp/T -----
        # slots largest-first so the exp/transpose pipeline overlaps B
        with tc.tile_pool(name="psB", bufs=4, space="PSUM") as psB:
            for m in (3, 2, 1, 0):
                s_t = s_pool.tile([128, BKT[m] * 128], f32, name=f"s{m}", tag="s")
                for kg in range(BG[m]):
                    ps = psB.tile([128, 512], f32, name="ps_s", tag="psb")
                    for kc in range(KC):
                        nc.tensor.matmul(
                            ps[:],
                            qt_sb[:, kc * RQ + m * 128 : kc * RQ + (m + 1) * 128],
                            xtp_t[kg][:, kc * 512 : (kc + 1) * 512],
                            start=(kc == 0),
                            stop=(kc == KC - 1),
                        )
                    dst = s_t[:, kg * 512 : (kg + 1) * 512]
                    if kg >= BG[m] - 2:
                        mk = kg - (BG[m] - 2)
                        nc.vector.tensor_add(
                            dst, ps[:], mask_t[m][:, mk * 512 : (mk + 1) * 512]
                        )
                    else:
                        nc.vector.tensor_copy(dst, ps[:])
                    nc.vector.tensor_reduce(
                        mpart[:, MPOFF[m] + kg : MPOFF[m] + kg + 1],
                        dst,
                        axis=mybir.AxisListType.X,
                        op=mybir.AluOpType.max,
                    )
                nc.vector.tensor_reduce(
                    negmax[:, m : m + 1],
                    mpart[:, MPOFF[m] : MPOFF[m] + BG[m]],
                    axis=mybir.AxisListType.X,
                    op=mybir.AluOpType.max,
                    negate=True,
                )
                # exp straight to fp8 attn; transpose the packed-fp16 view
                # through the xbar, one call per 1024-key block
                p_q = p_pool.tile(
                    [128, BKT[m] * 128], f8e4, name="p_q", tag="pq"
                )
                nc.scalar.activation(
                    p_q[:],
                    s_t[:],
                    mybir.ActivationFunctionType.Exp,
                    bias=negmax[:, m : m + 1],
                    scale=1.0,
                    accum_out=lsum[:, m : m + 1],
                )
                p16 = p_q[:].bitcast(f16)
                for c in range(NCH[m]):
                    nc.scalar.dma_start_transpose(
                        pt_view16(m, c),
                        p16[:, c * 512 : (c + 1) * 512],
                    )

        for m in range(4):
            nc.vector.reciprocal(recip[:, m : m + 1], lsum[:, m : m + 1])

        mask_pool.release()
        xtp_pool.release()
        qt_pool.release()
        s_pool.release()
        psW.release()

        # ---- Phase E: o1T[d] = sum over key pairs, fp8 DoubleRow ---------
        # o1 = attn8 @ (x_hi8 + x_lo8): attn is near-exact in e4m3 (softmax
        # is ~one-hot and exp(0)=1.0 is exact); x carries ~11-bit mantissa
        # via the hi+lo pair. Each matmul contracts a 256-key pairgroup at
        # 0.5 cyc/row (DoubleRow), halving E's tensor time vs fp16.
        wovstream = tc.alloc_tile_pool(name="wovstream", bufs=2)
        with tc.tile_pool(name="psE", bufs=1, space="PSUM") as psE_pool:
            psE = [
                psE_pool.tile([128, RQ], f32, name=f"psE{d}") for d in range(KC)
            ]
            # Blocks largest-key-index first: slots 1-3's attn lands during
            # B (largest-first slot order), so E starts right after B; the
            # slot0-only block 0 runs last, after slot0's post-B exp/xbar
            # (its xp buffer reuses block 3's, whose matmuls finish first).
            for bi, c in enumerate((3, 2, 1, 0)):
                xp_t = xpstream.tile(
                    [128, 2 * 8 * D], f8e4, name="xp_t", tag="xp"
                )
                for hl, src in enumerate((xph_d, xpl_d)):
                    nc.sync.dma_start(
                        xp_t[:, hl * 8 * D : (hl + 1) * 8 * D].rearrange(
                            "p (pg two n) -> p pg two n", pg=4, two=2
                        ),
                        src[c * 1024 : (c + 1) * 1024, :].rearrange(
                            "(pg two p) n -> p pg two n", pg=4, two=2
                        ),
                    )
                w = WC[c]
                # the final block runs d-major so each psum bank's chain
                # closes early and its evacuation overlaps E's tail
                if bi == 3:
                    pdh = [
                        (pg, d, hl)
                        for d in range(KC)
                        for pg in range(4)
                        for hl in range(2)
                    ]
                else:
                    pdh = [
                        (pg, d, hl)
                        for pg in range(4)
                        for d in range(KC)
                        for hl in range(2)
                    ]
                for pg, d, hl in pdh:
                    base = hl * 8 * D + pg * 2 * D
                    stat = xp_t[:, base : base + 2 * D].rearrange(
                        "p (two n) -> p two n", two=2
                    )[:, :, d * 128 : (d + 1) * 128]
                    mov = ptall[
                        :, PTOFF[c] + pg * 2 * w : PTOFF[c] + (pg + 1) * 2 * w
                    ].rearrange("p (q two) -> p two q", two=2)
                    # start_tensor_calc zeroes the WHOLE psum bank, so only
                    # the first matmul into bank d sets it; later slot
                    # regions accumulate onto zeros. All chains end in the
                    # final block (keys 0..1023).
                    nc.tensor.matmul(
                        psE[d][:, c * 128 : 512],
                        stat,
                        mov,
                        start=(bi == 0 and pg == 0 and hl == 0),
                        stop=(c == 0 and pg == 3 and hl == 1),
                        perf_mode=DR,
                        skip_group_check=True,
                    )
            # evacuate: split across DVE and Act so phase F starts sooner
            for d in range(KC):
                if d % 2 == 0:
                    nc.vector.tensor_copy(o1t[d][:], psE[d][:])
                else:
                    nc.scalar.activation(
                        o1t[d][:],
                        psE[d][:],
                        mybir.ActivationFunctionType.Copy,
                    )

        # ---- Phase F: out = (o1 @ Wov) * recip ---------------------------
        with (
            tc.tile_pool(name="psF", bufs=2, space="PSUM") as psF,
            tc.tile_pool(name="outp", bufs=3) as outp,
        ):
            for nb in range(2):
                wov_blk = wovstream.tile(
                    [128, KC * 512], f16, name="wov_blk", tag="wv"
                )
                nc.sync.dma_start(
                    wov_blk.rearrange("p (kc n) -> p kc n", kc=KC),
                    wov_d[:, nb * 512 : (nb + 1) * 512].rearrange(
                        "(kc p) n -> p kc n", p=128
                    ),
                )
                for m in range(4):
                    halves = 2 if (nb == 1 and m == 3) else 1
                    w = 512 // halves
                    for h in range(halves):
                        ps = psF.tile([128, w], f32, name="ps_o", tag="pso")
                        for kc in range(KC):
                            nc.tensor.matmul(
                                ps[:],
                                o1t[kc][:, m * 128 : (m + 1) * 128],
                                wov_blk[
                                    :, kc * 512 + h * w : kc * 512 + h * w + w
                                ],
                                start=(kc == 0),
                                stop=(kc == KC - 1),
                            )
                        ob = outp.tile([128, w], f16, name="ob", tag="ob")
                        nc.vector.tensor_scalar_mul(
                            ob[:], ps[:], recip[:, m : m + 1]
                        )
                        nc.sync.dma_start(
                            out_d[
                                m * 128 : (m + 1) * 128,
                                nb * 512 + h * w : nb * 512 + h * w + w,
                            ],
                            ob[:],
                        )

        wovstream.release()
        p_pool.release()
        xpstream.release()
        o1_pool.release()
        pt_pool.release()
        consts.release()

    nc.compile()
    return nc


_NC_CACHE = {}


def _get_nc():
    if "nc" not in _NC_CACHE:
        _NC_CACHE["nc"] = _build_nc()
    return _NC_CACHE["nc"]


def _slot_tiles(c):
    return [c, 15 - c, 16 + c, 31 - c]


def _prep_in_maps(x, Wqk, Wov):
    import ml_dtypes

    x = np.ascontiguousarray(np.asarray(x), dtype=np.float32)
    Wqk = np.ascontiguousarray(np.asarray(Wqk), dtype=np.float32)
    Wov = np.ascontiguousarray(np.asarray(Wov), dtype=np.float32)
    x16 = x.astype(np.float16)
    xT16 = np.ascontiguousarray(x16.T)  # [D, T]
    wqk16 = Wqk.astype(np.float16)
    wov16 = Wov.astype(np.float16)

    # x rows as fp8 hi + lo (hi+lo carries ~11-bit mantissa), rows permuted
    # so dram row 256g + 128i + p holds logical row 256g + 2p + i: after the
    # pairgroup DMA, partition p carries rows (2p, 2p+1) of its pairgroup as
    # DoubleRow contraction pairs.
    xh8 = x.astype(ml_dtypes.float8_e4m3)
    xl8 = (x - xh8.astype(np.float32)).astype(ml_dtypes.float8_e4m3)
    # dram row index (g, i, p): logical = 256g + 2p + i
    idx = (
        np.arange(T // 256)[:, None, None] * 256
        + 2 * np.arange(128)[None, None, :]
        + np.arange(2)[None, :, None]
    ).reshape(-1)
    xph = np.ascontiguousarray(xh8[idx])
    xpl = np.ascontiguousarray(xl8[idx])

    in_maps = []
    for c in range(NCORES):
        tiles = _slot_tiles(c)
        rows = np.concatenate(
            [np.arange(t * 128, (t + 1) * 128) for t in tiles]
        )
        xqt = np.ascontiguousarray(xT16[:, rows])

        mask = np.full((128, 4096), NEG16, dtype=ml_dtypes.float8_e5m2)
        p = np.arange(128)[:, None]
        for m, t in enumerate(tiles):
            g = t * 128 + p  # global row index per partition
            # last 1024 keys of slot m's budget: [1024m, 1024(m+1))
            y = 1024 * m + np.arange(1024)[None, :]
            mask[:, m * 1024 : (m + 1) * 1024] = np.where(
                y <= g, 0.0, NEG16
            ).astype(ml_dtypes.float8_e5m2)
        in_maps.append(
            {
                "xqt": xqt,
                "xtp": xT16,
                "xph": xph,
                "xpl": xpl,
                "wqk": wqk16,
                "wov": wov16,
                "mask": mask,
            }
        )
    return in_maps


def run(x, Wqk, Wov, **spmd_kwargs):
    """Full pipeline; returns (output [T, D] fp32, BassKernelResults)."""
    import time

    nc = _get_nc()
    in_maps = _prep_in_maps(x, Wqk, Wov)
    try:
        res = run_bass_kernel_spmd(
            nc, in_maps, core_ids=list(range(NCORES)), **spmd_kwargs
        )
    except Exception:
        # a prior crashed execution can leave a core transiently
        # unrecoverable; the runtime resets it — retry once
        time.sleep(10)
        res = run_bass_kernel_spmd(
            nc, in_maps, core_ids=list(range(NCORES)), **spmd_kwargs
        )
    out = np.empty((T, D), dtype=np.float32)
    for c in range(NCORES):
        co = res.results[c]["out"]
        for m, t in enumerate(_slot_tiles(c)):
            out[t * 128 : (t + 1) * 128] = co[m * 128 : (m + 1) * 128]
    return np.ascontiguousarray(out), res


def kernel(x, Wqk, Wov):
    out, _ = run(x, Wqk, Wov)
    return out



# revision 27
# speedup vs baseline: 1.0537x; 1.0537x over previous
"""Causal attention kernel for Trainium2, 8 NeuronCores, sequence-parallel.

Reference computation (T=4096, D=1024, fp32):
    q = x @ Wqk; logits = q @ x.T (causal masked); attn = softmax(logits)
    out = (attn @ x) @ Wov

Causal load balancing under one SPMD program: the 32 query row-tiles of 128
are assigned to cores as {c, 15-c, 16+c, 31-c} and host-permuted into 4
local "slots" ordered by visibility class. Slot m processes a fixed key
budget of 8*(m+1) key-tiles (keys in natural order, prefix [0, 1024*(m+1))),
which covers every core's visible range in that class. Causality inside the
budget is enforced by a host-provided additive mask (0 / -60000) that also
carries the diagonal triangle, so the program is core-independent while
skipping 37.5% of the score/AV matmul work.

Matmul precision: fp16 inputs (x, Wqk, Wov, attn) with fp32 PSUM
accumulation; q and o1 kept in fp16 on-chip. Softmax row max subtracted in
fp32; attn stored fp16 for the DMA-xbar transposes and AV.

Scheduling notes: input DMAs are issued in consumption order (xqt/wqk for
phase A first, then keys/masks); phase B runs slots largest-first so the
exp/transpose pipeline drains during B and phase E can start right after;
tiles are split per dependency unit (per-kg keys, per-chunk attn-transpose,
per-d o1) to keep cross-engine waits granular.
"""

import sys

sys.path.insert(0, "/opt/trn_rl_repo")

import numpy as np

import concourse.tile as tile
from concourse import bacc, mybir
from concourse.bass_utils import run_bass_kernel_spmd

T = 4096
D = 1024
NCORES = 8
RQ = T // NCORES  # 512 query rows per core
KC = D // 128  # 8 contraction chunks
NEG16 = -57344.0  # exactly representable in fp8e5m2

BKT = [8, 16, 24, 32]  # key tiles (128) processed per slot
BG = [b // 4 for b in BKT]  # 512-wide key groups per slot
OFFK = [0, 1024, 3072, 6144]  # slot column offsets in ragged score layout
STOT = 10240  # total score/mask columns
MPOFF = [0, 2, 6, 12]  # mpart offsets (prefix of BG)
NCH = [b // 8 for b in BKT]  # 1024-wide exp chunks per slot: 1,2,3,4
LQOFF = [0, 1, 3, 6]  # lq offsets (prefix of NCH)

f32 = mybir.dt.float32
f16 = mybir.dt.float16
f8 = mybir.dt.float8e5
f8e4 = mybir.dt.float8e4
DR = mybir.MatmulPerfMode.DoubleRow


def _build_nc():
    nc = bacc.Bacc(
        "TRN2", target_bir_lowering=False, debug=False, num_devices=NCORES
    )

    xqt_d = nc.dram_tensor("xqt", [D, RQ], f16, kind="ExternalInput").ap()
    xtp_d = nc.dram_tensor("xtp", [D, T], f16, kind="ExternalInput").ap()
    # x rows in fp8 hi/lo, host-permuted so partition p of a 256-row
    # pairgroup holds rows 2p and 2p+1 (DoubleRow contraction pairs)
    xph_d = nc.dram_tensor("xph", [T, D], f8e4, kind="ExternalInput").ap()
    xpl_d = nc.dram_tensor("xpl", [T, D], f8e4, kind="ExternalInput").ap()
    wqk_d = nc.dram_tensor("wqk", [D, D], f16, kind="ExternalInput").ap()
    wov_d = nc.dram_tensor("wov", [D, D], f16, kind="ExternalInput").ap()
    ident_d = nc.dram_tensor("ident", [128, 128], f16, kind="ExternalInput").ap()
    # causal mask, only the last 1024 keys of each slot's budget: for class
    # m (tiles 8m..8m+7) keys below 1024m are visible to every member core,
    # so only the final window carries the triangle / -inf region
    mask_d = nc.dram_tensor("mask", [128, 4096], f8, kind="ExternalInput").ap()
    out_d = nc.dram_tensor("out", [RQ, D], f16, kind="ExternalOutput").ap()

    with tile.TileContext(nc) as tc:
        # stack allocator: long-lived pools first
        consts = tc.alloc_tile_pool(name="consts", bufs=1)
        pt_pool = tc.alloc_tile_pool(name="ptpool", bufs=1)
        o1_pool = tc.alloc_tile_pool(name="o1pool", bufs=1)
        xpstream = tc.alloc_tile_pool(name="xpstream", bufs=3)
        p_pool = tc.alloc_tile_pool(name="ppool", bufs=2)
        s_pool = tc.alloc_tile_pool(name="spool", bufs=2)
        qt_pool = tc.alloc_tile_pool(name="qt", bufs=1)
        xtp_pool = tc.alloc_tile_pool(name="xtpp", bufs=1)
        mask_pool = tc.alloc_tile_pool(name="maskp", bufs=1)
        wqk_pool = tc.alloc_tile_pool(name="wqkp", bufs=1)
        xqt_pool = tc.alloc_tile_pool(name="xqtp", bufs=1)

        # stats scratch: negmax 0:4, lsum 4:8, recip 8:12, mpart 12:32, lq 32:42
        smalls = consts.tile([128, 48], f32, name="smalls")
        dum = consts.tile([128, 256], f16, name="dum")
        ident = consts.tile([128, 128], f16, name="ident")
        negmax = smalls[:, 0:4]
        lsum = smalls[:, 4:8]
        recip = smalls[:, 8:12]
        mpart = smalls[:, 12:32]
        lq = smalls[:, 32:42]

        # transposed attn, fp8, packed in key-PAIRS: the xbar moves 2-byte
        # granules, so attn fp8 is transposed as an fp16 view — partition p
        # of a 256-key pairgroup then holds keys (2p, 2p+1) as adjacent
        # bytes, exactly the [p, 2, q] moving layout DoubleRow contracts.
        # Ragged blocks: block c (pairgroups 4c..4c+3) has q-width
        # (4-c)*128 covering slots m >= c; byte offsets PTOFF.
        PTOFF = [0, 4096, 7168, 9216]  # byte prefix sums of 4*2*width(c)
        WC = [512, 384, 256, 128]  # q-cols per block
        ptall = pt_pool.tile([128, STOT], f8e4, name="ptall")

        def pt_view16(m, c):
            # [128, 4 pairgroups, 128] fp16 xbar-dst view of slot m, block c
            w = WC[c]
            span = ptall[:].bitcast(f16)[
                :, PTOFF[c] // 2 : PTOFF[c] // 2 + 4 * w
            ].rearrange("p (pg w) -> p pg w", pg=4)
            return span[:, :, (m - c) * 128 : (m - c) * 128 + 128]
        o1t = [o1_pool.tile([128, RQ], f16, name=f"o1t{d}") for d in range(KC)]
        qt_sb = qt_pool.tile([128, KC * RQ], f16, name="qt_sb")
        xtp_t = [
            xtp_pool.tile([128, KC * 512], f16, name=f"xtp{kg}")
            for kg in range(T // 512)
        ]
        mask_t = [
            mask_pool.tile([128, 1024], f8, name=f"mask{m}") for m in range(4)
        ]
        wqk_t = [
            wqk_pool.tile([128, KC * 256], f16, name=f"wqk{md2}")
            for md2 in range(KC // 2)
        ]
        xqt_sb = xqt_pool.tile([128, KC * RQ], f16, name="xqt_sb")

        # ---- input DMAs, issued in consumption order ---------------------
        def load_wqk(md2):
            nc.sync.dma_start(
                wqk_t[md2].rearrange("p (kc n) -> p kc n", kc=KC),
                wqk_d[:, md2 * 256 : (md2 + 1) * 256].rearrange(
                    "(kc p) n -> p kc n", p=128
                ),
            )

        def load_xtp(kg):
            nc.sync.dma_start(
                xtp_t[kg].rearrange("p (kc n) -> p kc n", kc=KC),
                xtp_d[:, kg * 512 : (kg + 1) * 512].rearrange(
                    "(kc p) n -> p kc n", p=128
                ),
            )

        def load_mask(m):
            nc.sync.dma_start(
                mask_t[m], mask_d[:, m * 1024 : (m + 1) * 1024]
            )

        nc.sync.dma_start(
            xqt_sb.rearrange("p (kc n) -> p kc n", kc=KC),
            xqt_d.rearrange("(kc p) n -> p kc n", p=128),
        )
        load_wqk(0)
        load_wqk(1)
        load_wqk(2)
        load_xtp(0)
        load_wqk(3)
        load_xtp(1)
        load_xtp(2)
        load_xtp(3)
        load_mask(3)
        for kg in range(4, 8):
            load_xtp(kg)
        load_mask(2)
        load_mask(1)
        load_mask(0)
        nc.sync.dma_start(ident, ident_d)

        # PE p-state warmup: the tensor engine downclocks when idle and
        # takes ~3us to re-ramp. Keep it hot with throwaway matmuls into a
        # dedicated PSUM bank while input DMAs land / cross-engine deps
        # resolve. psW is allocated first so its WAR chains stay PE-internal.
        psW = tc.alloc_tile_pool(name="psW", bufs=1, space="PSUM")
        wps = psW.tile([128, 512], f32, name="wps")
        nc.gpsimd.memset(dum[:], 0.0)

        def warm(n):
            for _ in range(n):
                nc.tensor.matmul(
                    wps[:, 0:256], dum[:, 0:128], dum[:], start=True, stop=True
                )

        warm(18)

        # ---- Phase A: qT = (xq @ Wqk)^T  -> [D, RQ] fp16 -----------------
        with tc.tile_pool(name="psA", bufs=2, space="PSUM") as psA:
            for md2 in range(KC // 2):
                for h in range(2):
                    mtd = md2 * 2 + h
                    ps = psA.tile([128, RQ], f32, name="ps_qt")
                    for kc in range(KC):
                        nc.tensor.matmul(
                            ps[:],
                            wqk_t[md2][
                                :, kc * 256 + h * 128 : kc * 256 + h * 128 + 128
                            ],
                            xqt_sb[:, kc * RQ : (kc + 1) * RQ],
                            start=(kc == 0),
                            stop=(kc == KC - 1),
                        )
                    nc.vector.tensor_copy(
                        qt_sb[:, mtd * RQ : (mtd + 1) * RQ], ps[:]
                    )
        xqt_pool.release()
        wqk_pool.release()

        # ---- Phase B: per-slot scores + mask + softmax stats + exp/T -----
        # slots largest-first so the exp/transpose pipeline overlaps B
        p_q_late = {}
        with tc.tile_pool(name="psB", bufs=4, space="PSUM") as psB:
            for m in (3, 2, 1, 0):
                s_t = s_pool.tile([128, BKT[m] * 128], f32, name=f"s{m}", tag="s")
                for kg in range(BG[m]):
                    ps = psB.tile([128, 512], f32, name="ps_s", tag="psb")
                    for kc in range(KC):
                        nc.tensor.matmul(
                            ps[:],
                            qt_sb[:, kc * RQ + m * 128 : kc * RQ + (m + 1) * 128],
                            xtp_t[kg][:, kc * 512 : (kc + 1) * 512],
                            start=(kc == 0),
                            stop=(kc == KC - 1),
                        )
                    dst = s_t[:, kg * 512 : (kg + 1) * 512]
                    if kg >= BG[m] - 2:
                        mk = kg - (BG[m] - 2)
                        nc.vector.tensor_add(
                            dst, ps[:], mask_t[m][:, mk * 512 : (mk + 1) * 512]
                        )
                    else:
                        nc.vector.tensor_copy(dst, ps[:])
                    nc.vector.tensor_reduce(
                        mpart[:, MPOFF[m] + kg : MPOFF[m] + kg + 1],
                        dst,
                        axis=mybir.AxisListType.X,
                        op=mybir.AluOpType.max,
                    )
                nc.vector.tensor_reduce(
                    negmax[:, m : m + 1],
                    mpart[:, MPOFF[m] : MPOFF[m] + BG[m]],
                    axis=mybir.AxisListType.X,
                    op=mybir.AluOpType.max,
                    negate=True,
                )
                # exp straight to fp8 attn; transpose the packed-fp16 view
                # through the xbar, one call per 1024-key block
                p_q = p_pool.tile(
                    [128, BKT[m] * 128], f8e4, name="p_q", tag="pq"
                )
                nc.scalar.activation(
                    p_q[:],
                    s_t[:],
                    mybir.ActivationFunctionType.Exp,
                    bias=negmax[:, m : m + 1],
                    scale=1.0,
                    accum_out=lsum[:, m : m + 1],
                )
                # slots 3/2 transpose through the xbar during B; slots 1/0
                # finish after the xp-prefetch flood occupies the (serial)
                # DMA engine, so they transpose on the PE below instead
                if m >= 2:
                    p16 = p_q[:].bitcast(f16)
                    for c in range(NCH[m]):
                        nc.scalar.dma_start_transpose(
                            pt_view16(m, c),
                            p16[:, c * 512 : (c + 1) * 512],
                        )
                else:
                    p_q_late[m] = p_q

        for m in range(4):
            nc.vector.reciprocal(recip[:, m : m + 1], lsum[:, m : m + 1])

        mask_pool.release()
        xtp_pool.release()
        qt_pool.release()
        s_pool.release()

        # slots 1/0: transpose the packed-fp16 view on the PE (one [128,128]
        # matmul per 256-key pairgroup) into the warmup PSUM bank viewed as
        # fp16, evacuated by the otherwise-idle DVE into ptall.
        wps16 = wps[:].bitcast(f16)  # [128, 1024]

        def pe_transpose(m):
            p16 = p_q_late[m][:].bitcast(f16)
            npg = BKT[m] // 2
            for pgg in range(npg):
                nc.tensor.matmul(
                    wps16[:, pgg * 128 : (pgg + 1) * 128],
                    p16[:, pgg * 128 : (pgg + 1) * 128],
                    ident,
                    is_transpose=True,
                    start=(pgg == 0),
                    stop=(pgg == npg - 1),
                    skip_group_check=True,
                )
            for c in range(NCH[m]):
                nc.vector.tensor_copy(
                    pt_view16(m, c),
                    wps16[:, c * 512 : (c + 1) * 512].rearrange(
                        "p (pg w) -> p pg w", pg=4
                    ),
                )

        with tc.tile_pool(name="psbr", bufs=1, space="PSUM") as psbr:
            wbr = psbr.tile([128, 512], f32, name="wbr")
            pe_transpose(1)
            # bridge slot0's exp latency with warmup matmuls into a free
            # bank (not the transpose bank: evac reads are still in flight)
            for _ in range(8):
                nc.tensor.matmul(
                    wbr[:, 0:256], dum[:, 0:128], dum[:], start=True, stop=True
                )
            pe_transpose(0)
        psW.release()

        # ---- Phase E: o1T[d] = sum over key pairs, fp8 DoubleRow ---------
        # o1 = attn8 @ (x_hi8 + x_lo8): attn is near-exact in e4m3 (softmax
        # is ~one-hot and exp(0)=1.0 is exact); x carries ~11-bit mantissa
        # via the hi+lo pair. Each matmul contracts a 256-key pairgroup at
        # 0.5 cyc/row (DoubleRow), halving E's tensor time vs fp16.
        wovstream = tc.alloc_tile_pool(name="wovstream", bufs=2)
        with tc.tile_pool(name="psE", bufs=1, space="PSUM") as psE_pool:
            psE = [
                psE_pool.tile([128, RQ], f32, name=f"psE{d}") for d in range(KC)
            ]
            # Blocks largest-key-index first: slots 1-3's attn lands during
            # B (largest-first slot order), so E starts right after B; the
            # slot0-only block 0 runs last, after slot0's post-B exp/xbar
            # (its xp buffer reuses block 3's, whose matmuls finish first).
            for bi, c in enumerate((3, 2, 1, 0)):
                xp_t = xpstream.tile(
                    [128, 2 * 8 * D], f8e4, name="xp_t", tag="xp"
                )
                for hl, src in enumerate((xph_d, xpl_d)):
                    nc.sync.dma_start(
                        xp_t[:, hl * 8 * D : (hl + 1) * 8 * D].rearrange(
                            "p (pg two n) -> p pg two n", pg=4, two=2
                        ),
                        src[c * 1024 : (c + 1) * 1024, :].rearrange(
                            "(pg two p) n -> p pg two n", pg=4, two=2
                        ),
                    )
                w = WC[c]
                # the final block runs d-major so each psum bank's chain
                # closes early and its evacuation overlaps E's tail
                if bi == 3:
                    pdh = [
                        (pg, d, hl)
                        for d in range(KC)
                        for pg in range(4)
                        for hl in range(2)
                    ]
                else:
                    pdh = [
                        (pg, d, hl)
                        for pg in range(4)
                        for d in range(KC)
                        for hl in range(2)
                    ]
                for pg, d, hl in pdh:
                    base = hl * 8 * D + pg * 2 * D
                    stat = xp_t[:, base : base + 2 * D].rearrange(
                        "p (two n) -> p two n", two=2
                    )[:, :, d * 128 : (d + 1) * 128]
                    mov = ptall[
                        :, PTOFF[c] + pg * 2 * w : PTOFF[c] + (pg + 1) * 2 * w
                    ].rearrange("p (q two) -> p two q", two=2)
                    # start_tensor_calc zeroes the WHOLE psum bank, so only
                    # the first matmul into bank d sets it; later slot
                    # regions accumulate onto zeros. All chains end in the
                    # final block (keys 0..1023).
                    nc.tensor.matmul(
                        psE[d][:, c * 128 : 512],
                        stat,
                        mov,
                        start=(bi == 0 and pg == 0 and hl == 0),
                        stop=(c == 0 and pg == 3 and hl == 1),
                        perf_mode=DR,
                        skip_group_check=True,
                    )
            # evacuate: split across DVE and Act so phase F starts sooner
            for d in range(KC):
                if d % 2 == 0:
                    nc.vector.tensor_copy(o1t[d][:], psE[d][:])
                else:
                    nc.scalar.activation(
                        o1t[d][:],
                        psE[d][:],
                        mybir.ActivationFunctionType.Copy,
                    )

        # ---- Phase F: out = (o1 @ Wov) * recip ---------------------------
        with (
            tc.tile_pool(name="psF", bufs=2, space="PSUM") as psF,
            tc.tile_pool(name="outp", bufs=3) as outp,
        ):
            for nb in range(2):
                wov_blk = wovstream.tile(
                    [128, KC * 512], f16, name="wov_blk", tag="wv"
                )
                nc.sync.dma_start(
                    wov_blk.rearrange("p (kc n) -> p kc n", kc=KC),
                    wov_d[:, nb * 512 : (nb + 1) * 512].rearrange(
                        "(kc p) n -> p kc n", p=128
                    ),
                )
                for m in range(4):
                    halves = 2 if (nb == 1 and m == 3) else 1
                    w = 512 // halves
                    for h in range(halves):
                        ps = psF.tile([128, w], f32, name="ps_o", tag="pso")
                        for kc in range(KC):
                            nc.tensor.matmul(
                                ps[:],
                                o1t[kc][:, m * 128 : (m + 1) * 128],
                                wov_blk[
                                    :, kc * 512 + h * w : kc * 512 + h * w + w
                                ],
                                start=(kc == 0),
                                stop=(kc == KC - 1),
                            )
                        ob = outp.tile([128, w], f16, name="ob", tag="ob")
                        nc.vector.tensor_scalar_mul(
                            ob[:], ps[:], recip[:, m : m + 1]
                        )
                        nc.sync.dma_start(
                            out_d[
                                m * 128 : (m + 1) * 128,
                                nb * 512 + h * w : nb * 512 + h * w + w,
                            ],
                            ob[:],
                        )

        wovstream.release()
        p_pool.release()
        xpstream.release()
        o1_pool.release()
        pt_pool.release()
        consts.release()

    nc.compile()
    return nc


_NC_CACHE = {}


def _get_nc():
    if "nc" not in _NC_CACHE:
        _NC_CACHE["nc"] = _build_nc()
    return _NC_CACHE["nc"]


def _slot_tiles(c):
    return [c, 15 - c, 16 + c, 31 - c]


def _prep_in_maps(x, Wqk, Wov):
    import ml_dtypes

    x = np.ascontiguousarray(np.asarray(x), dtype=np.float32)
    Wqk = np.ascontiguousarray(np.asarray(Wqk), dtype=np.float32)
    Wov = np.ascontiguousarray(np.asarray(Wov), dtype=np.float32)
    x16 = x.astype(np.float16)
    xT16 = np.ascontiguousarray(x16.T)  # [D, T]
    wqk16 = Wqk.astype(np.float16)
    wov16 = Wov.astype(np.float16)

    # x rows as fp8 hi + lo (hi+lo carries ~11-bit mantissa), rows permuted
    # so dram row 256g + 128i + p holds logical row 256g + 2p + i: after the
    # pairgroup DMA, partition p carries rows (2p, 2p+1) of its pairgroup as
    # DoubleRow contraction pairs.
    xh8 = x.astype(ml_dtypes.float8_e4m3)
    xl8 = (x - xh8.astype(np.float32)).astype(ml_dtypes.float8_e4m3)
    # dram row index (g, i, p): logical = 256g + 2p + i
    idx = (
        np.arange(T // 256)[:, None, None] * 256
        + 2 * np.arange(128)[None, None, :]
        + np.arange(2)[None, :, None]
    ).reshape(-1)
    xph = np.ascontiguousarray(xh8[idx])
    xpl = np.ascontiguousarray(xl8[idx])

    in_maps = []
    for c in range(NCORES):
        tiles = _slot_tiles(c)
        rows = np.concatenate(
            [np.arange(t * 128, (t + 1) * 128) for t in tiles]
        )
        xqt = np.ascontiguousarray(xT16[:, rows])

        mask = np.full((128, 4096), NEG16, dtype=ml_dtypes.float8_e5m2)
        p = np.arange(128)[:, None]
        for m, t in enumerate(tiles):
            g = t * 128 + p  # global row index per partition
            # last 1024 keys of slot m's budget: [1024m, 1024(m+1))
            y = 1024 * m + np.arange(1024)[None, :]
            mask[:, m * 1024 : (m + 1) * 1024] = np.where(
                y <= g, 0.0, NEG16
            ).astype(ml_dtypes.float8_e5m2)
        in_maps.append(
            {
                "xqt": xqt,
                "xtp": xT16,
                "xph": xph,
                "xpl": xpl,
                "wqk": wqk16,
                "wov": wov16,
                "mask": mask,
                "ident": np.eye(128, dtype=np.float16),
            }
        )
    return in_maps


def run(x, Wqk, Wov, **spmd_kwargs):
    """Full pipeline; returns (output [T, D] fp32, BassKernelResults)."""
    import time

    nc = _get_nc()
    in_maps = _prep_in_maps(x, Wqk, Wov)
    try:
        res = run_bass_kernel_spmd(
            nc, in_maps, core_ids=list(range(NCORES)), **spmd_kwargs
        )
    except Exception:
        # a prior crashed execution can leave a core transiently
        # unrecoverable; the runtime resets it — retry once
        time.sleep(10)
        res = run_bass_kernel_spmd(
            nc, in_maps, core_ids=list(range(NCORES)), **spmd_kwargs
        )
    out = np.empty((T, D), dtype=np.float32)
    for c in range(NCORES):
        co = res.results[c]["out"]
        for m, t in enumerate(_slot_tiles(c)):
            out[t * 128 : (t + 1) * 128] = co[m * 128 : (m + 1) * 128]
    return np.ascontiguousarray(out), res


def kernel(x, Wqk, Wov):
    out, _ = run(x, Wqk, Wov)
    return out

